# revision 32
# baseline (speedup 1.0000x reference)
# Trainium2 Bass kernel for nn_Actor (gnn_message_passing), 8-core data parallel.
#
# Math (per batch row b):
#   att = sigmoid(g @ W_cast + b_cast)                      [50]
#   x_n = concat(body(20), obj_n(30)) * att                 [50] per object n<8
#   h_n = relu(x_n @ W_a0 + b_a0)                           [256]
#   y_n = relu(h_n @ W_a1 + b_a1)                           [400]
#   pi  = sum_n y_n                                         [400]
#   out = tanh(relu(relu(pi@W_p0+b_p0)@W_p1+b_p1)@W_p2+b_p2)  [8]
#
# Mapping: everything feature-major on-chip ([feature partitions, batch free]).
# Host pre-transposes/gathers o into xsrc pair tiles (objects 2p / 2p+1 at
# partitions 0:51 / 64:115, row 50/114 is a constant-1 row that folds b_a0
# into the a0 matmul; W_cast is column-replicated so the attention PSUM
# comes out already in the gate layout). bf16 matmuls, fp32 PSUM.
import numpy as np
import ml_dtypes

BF16 = ml_dtypes.bfloat16

B = 65536
NCORES = 8
BSH = B // NCORES          # 8192 rows per core
TN = 512                   # batch tile (matmul free dim / psum bank)
BODY = 10
FEAT = 15
NOBJ = 8
HALF = 130

_BODY_COLS = list(range(0, 10)) + list(range(130, 140))


def _obj_cols(n):
    return list(range(10 + 15 * n, 25 + 15 * n)) + list(range(140 + 15 * n, 155 + 15 * n))


# ---------------------------------------------------------------- host packing

def _pack_weights(W_cast, b_cast, W_a0, b_a0, W_a1, b_a1,
                  W_p0, b_p0, W_p1, b_p1, W_p2, b_p2):
    f32 = np.float32
    # attention weights, column-replicated into the gate layout:
    # psum rows 0:50 = att[0:50] (object A gate), rows 64:114 = same (object B),
    # row 50/114 -> constant 1.0 after sigmoid (bias 30).
    wcast = np.zeros((100, 128), f32)
    wcast[:, 0:50] = W_cast
    wcast[:, 64:114] = W_cast
    bcast = np.zeros((128, 1), f32)
    bcast[0:50, 0] = b_cast
    bcast[64:114, 0] = b_cast
    bcast[50, 0] = 30.0
    bcast[114, 0] = 30.0

    # a0: K=51 (50 features + const-1 row carrying b_a0), M=256 in two chunks.
    wa0 = np.zeros((128, 2, 128), f32)
    for c in range(2):
        wa0[0:50, c, :] = W_a0[:, 128 * c:128 * (c + 1)]
        wa0[50, c, :] = b_a0[128 * c:128 * (c + 1)]
        wa0[64:114, c, :] = W_a0[:, 128 * c:128 * (c + 1)]
        wa0[114, c, :] = b_a0[128 * c:128 * (c + 1)]

    # a1: last M chunk (cols 384:512 of the packed slot) holds y[384:400] at
    # psum rows 1:17; row 0 is reserved for the p0 const-bias row.
    wa1 = np.zeros((128, 2, 512), f32)
    wa1[:, 0, 0:384] = W_a1[0:128, 0:384]
    wa1[:, 1, 0:384] = W_a1[128:256, 0:384]
    wa1[:, 0, 385:401] = W_a1[0:128, 384:400]
    wa1[:, 1, 385:401] = W_a1[128:256, 384:400]
    ba1 = np.zeros((128, 4), f32)
    ba1[:, 0] = b_a1[0:128]
    ba1[:, 1] = b_a1[128:256]
    ba1[:, 2] = b_a1[256:384]
    ba1[1:17, 3] = b_a1[384:400]

    # p0: K=400 as (128,128,128,17) — row 0 of last chunk is const-1 * b_p0.
    wp0 = np.zeros((128, 4, 256), f32)
    wp0[:, 0, :] = W_p0[0:128, :]
    wp0[:, 1, :] = W_p0[128:256, :]
    wp0[:, 2, :] = W_p0[256:384, :]
    wp0[0, 3, :] = b_p0
    wp0[1:17, 3, :] = W_p0[384:400, :]

    wp1 = np.zeros((128, 2, 256), f32)
    wp1[:, 0, :] = W_p1[0:128, :]
    wp1[:, 1, :] = W_p1[128:256, :]
    bp1 = np.zeros((128, 2), f32)
    bp1[:, 0] = b_p1[0:128]
    bp1[:, 1] = b_p1[128:256]

    wp2 = np.zeros((128, 2, 8), f32)
    wp2[:, 0, :] = W_p2[0:128, :]
    wp2[:, 1, :] = W_p2[128:256, :]
    bp2 = np.asarray(b_p2, f32).reshape(8, 1)

    return {
        "wcast": wcast.astype(BF16), "bcast": bcast,
        "wa0": wa0.astype(BF16),
        "wa1": wa1.astype(BF16), "ba1": ba1,
        "wp0": wp0.astype(BF16),
        "wp1": wp1.astype(BF16), "bp1": bp1,
        "wp2": wp2.astype(BF16), "bp2": bp2,
    }


def _pack_shard(o_sh, g_sh):
    """o_sh [bsh, 260] f32, g_sh [bsh, 100] f32 -> xsrc [4,128,bsh], gT [100,bsh] bf16."""
    bsh = o_sh.shape[0]
    oT = np.ascontiguousarray(o_sh.T)  # [260, bsh]
    xsrc = np.zeros((4, 128, bsh), np.float32)
    body = oT[_BODY_COLS]  # [20, bsh]
    for p in range(4):
        xsrc[p, 0:20] = body
        xsrc[p, 20:50] = oT[_obj_cols(2 * p)]
        xsrc[p, 50] = 1.0
        xsrc[p, 64:84] = body
        xsrc[p, 84:114] = oT[_obj_cols(2 * p + 1)]
        xsrc[p, 114] = 1.0
    gT = np.ascontiguousarray(g_sh.T)  # [100, bsh]
    return {"xsrc": xsrc.astype(BF16), "gT": gT.astype(BF16)}


# ---------------------------------------------------------------- bass kernel

def build_nc(bsh=BSH, zero_ba1=True, zero_bp1=True):
    import concourse.bass as bass
    import concourse.mybir as mybir
    import concourse.tile as tile
    from concourse import bacc

    f32 = mybir.dt.float32
    bf16 = mybir.dt.bfloat16
    AF = mybir.ActivationFunctionType
    ALU = mybir.AluOpType

    nt = bsh // TN
    nc = bacc.Bacc("TRN2", target_bir_lowering=False, debug=False)

    xsrc_d = nc.dram_tensor("xsrc", [4, 128, bsh], bf16, kind="ExternalInput")
    gT_d = nc.dram_tensor("gT", [100, bsh], bf16, kind="ExternalInput")
    wcast_d = nc.dram_tensor("wcast", [100, 128], bf16, kind="ExternalInput")
    bcast_d = nc.dram_tensor("bcast", [128, 1], f32, kind="ExternalInput")
    wa0_d = nc.dram_tensor("wa0", [128, 2, 128], bf16, kind="ExternalInput")
    wa1_d = nc.dram_tensor("wa1", [128, 2, 512], bf16, kind="ExternalInput")
    ba1_d = nc.dram_tensor("ba1", [128, 4], f32, kind="ExternalInput")
    wp0_d = nc.dram_tensor("wp0", [128, 4, 256], bf16, kind="ExternalInput")
    wp1_d = nc.dram_tensor("wp1", [128, 2, 256], bf16, kind="ExternalInput")
    bp1_d = nc.dram_tensor("bp1", [128, 2], f32, kind="ExternalInput")
    wp2_d = nc.dram_tensor("wp2", [128, 2, 8], bf16, kind="ExternalInput")
    bp2_d = nc.dram_tensor("bp2", [8, 1], f32, kind="ExternalInput")
    out_d = nc.dram_tensor("out", [8, bsh], f32, kind="ExternalOutput")

    MC = [(0, 128), (128, 128), (256, 128), (384, 16)]  # a1/p0 chunking of 400

    with tile.TileContext(nc) as tc:
        with (
            tc.tile_pool(name="s_w", bufs=1) as s_w,
            tc.tile_pool(name="s_in", bufs=3) as s_in,
            tc.tile_pool(name="s_x", bufs=2) as s_x,
            tc.tile_pool(name="s_h", bufs=5) as s_h,
            tc.tile_pool(name="s_t", bufs=4) as s_t,
            tc.tile_pool(name="s_pi", bufs=3) as s_pi,
            tc.tile_pool(name="s_p", bufs=2) as s_p,
            tc.tile_pool(name="s_o", bufs=2) as s_o,
            tc.tile_pool(name="p_h", bufs=1, space="PSUM") as p_h,
            tc.tile_pool(name="p_y", bufs=1, space="PSUM") as p_y,
        ):
            # --- load weights once
            wcast = s_w.tile([100, 128], bf16, tag="wcast")
            nc.sync.dma_start(wcast[:], wcast_d[:, :])
            bcast = s_w.tile([128, 1], f32, tag="bcast")
            nc.sync.dma_start(bcast[:], bcast_d[:, :])
            wa0 = s_w.tile([128, 2, 128], bf16, tag="wa0")
            nc.sync.dma_start(wa0[:], wa0_d[:, :, :])
            wa1 = s_w.tile([128, 2, 512], bf16, tag="wa1")
            nc.sync.dma_start(wa1[:], wa1_d[:, :, :])
            ba1 = s_w.tile([128, 4], f32, tag="ba1")
            nc.sync.dma_start(ba1[:], ba1_d[:, :])
            wp0 = s_w.tile([128, 4, 256], bf16, tag="wp0")
            nc.sync.dma_start(wp0[:], wp0_d[:, :, :])
            wp1 = s_w.tile([128, 2, 256], bf16, tag="wp1")
            nc.sync.dma_start(wp1[:], wp1_d[:, :, :])
            bp1 = s_w.tile([128, 2], f32, tag="bp1")
            nc.sync.dma_start(bp1[:], bp1_d[:, :])
            wp2 = s_w.tile([128, 2, 8], bf16, tag="wp2")
            nc.sync.dma_start(wp2[:], wp2_d[:, :, :])
            bp2 = s_w.tile([8, 1], f32, tag="bp2")
            nc.sync.dma_start(bp2[:], bp2_d[:, :])

            pi_store = {}    # t -> (piA, piB)
            p0T_store = {}   # t -> p0T
            p1T_store = {}   # t -> p1T
            xg_store = {}    # t -> [xg0..xg3]
            hT_store = {}    # (t, n) -> hT

            # ---- deferred p-chain emitters: tile t's tail is emitted during
            # ---- tile t+2, so pi/p0T/p1T latencies are fully hidden.
            def emit_p0(t):
                pi = pi_store.pop(t)
                p0_ps = p_y.tile([128, 2, TN], f32, tag="yA")
                for c in range(2):
                    mw = bass.ds(128 * c, 128)
                    nc.tensor.matmul(p0_ps[:, c, :], wp0[:, 0, mw], pi[:, 0, :],
                                     start=True, stop=False)
                    nc.tensor.matmul(p0_ps[:, c, :], wp0[:, 1, mw], pi[:, 1, :],
                                     start=False, stop=False)
                    nc.tensor.matmul(p0_ps[:, c, :], wp0[:, 2, mw], pi[:, 2, :],
                                     start=False, stop=False)
                    nc.tensor.matmul(p0_ps[:, c, :], wp0[0:17, 3, mw],
                                     pi[0:17, 3, :], start=False, stop=True)
                p0T = s_p.tile([128, 2, TN], bf16, tag="p0T")
                nc.scalar.activation(p0T[:], p0_ps[:], AF.Relu)
                p0T_store[t] = p0T

            def emit_p1(t):
                p0T = p0T_store.pop(t)
                p1_ps = p_h.tile([128, 2, TN], f32, tag="h")
                for c in range(2):
                    mw = bass.ds(128 * c, 128)
                    nc.tensor.matmul(p1_ps[:, c, :], wp1[:, 0, mw], p0T[:, 0, :],
                                     start=True, stop=False)
                    nc.tensor.matmul(p1_ps[:, c, :], wp1[:, 1, mw], p0T[:, 1, :],
                                     start=False, stop=True)
                p1T = s_p.tile([128, 2, TN], bf16, tag="p1T")
                if zero_bp1:
                    nc.scalar.activation(p1T[:], p1_ps[:], AF.Relu)
                else:
                    nc.scalar.activation(p1T[:, 0, :], p1_ps[:, 0, :], AF.Relu,
                                         bias=bp1[:, 0:1])
                    nc.scalar.activation(p1T[:, 1, :], p1_ps[:, 1, :], AF.Relu,
                                         bias=bp1[:, 1:2])
                p1T_store[t] = p1T

            def emit_p2(t):
                p1T = p1T_store.pop(t)
                cs = bass.ds(t * TN, TN)
                o_ps = p_h.tile([8, TN], f32, tag="h")
                nc.tensor.matmul(o_ps[:], wp2[:, 0, :], p1T[:, 0, :],
                                 start=True, stop=False)
                nc.tensor.matmul(o_ps[:], wp2[:, 1, :], p1T[:, 1, :],
                                 start=False, stop=True)
                ot = s_o.tile([8, TN], f32, tag="ot")
                nc.scalar.activation(ot[:], o_ps[:], AF.Tanh, bias=bp2[:, 0:1])
                nc.sync.dma_start(out_d[:, cs], ot[:])

            # ---- attention/gating for tile t (emitted one tile ahead so the
            # ---- att->sigmoid->gating chain never gates the object stream)
            def emit_att_block(t):
                cs = bass.ds(t * TN, TN)
                xs = []
                for p in range(4):
                    xp = s_in.tile([128, TN], bf16, tag=f"xs{p}")
                    nc.sync.dma_start(xp[:], xsrc_d[p, :, cs])
                    xs.append(xp)
                gt = s_in.tile([100, TN], bf16, tag="gt")
                nc.sync.dma_start(gt[:], gT_d[:, cs])
                att_ps = p_y.tile([128, TN], f32, tag="yB")
                nc.tensor.matmul(att_ps[:], wcast[:], gt[:], start=True,
                                 stop=True)
                attr = s_x.tile([128, TN], bf16, tag="attr")
                nc.scalar.activation(attr[:], att_ps[:], AF.Sigmoid,
                                     bias=bcast[:, 0:1])
                xg = []
                for p in range(4):
                    xgp = s_x.tile([128, TN], bf16, tag=f"xg{p}")
                    nc.gpsimd.tensor_tensor(xgp[:], xs[p][:], attr[:], ALU.mult)
                    xg.append(xgp)
                xg_store[t] = xg

            def emit_a0_pair(t, p):
                # objects 2p (rows 0:51) and 2p+1 (rows 64:115): alternate the
                # row-group halves so consecutive matmuls run concurrently on
                # disjoint PE quadrants; one 4-bank psum tile so Tile emits no
                # semaphore wait between the pair's matmuls.
                xg = xg_store[t]
                h_ps = p_h.tile([128, 4, TN], f32, tag="h")
                for c in range(2):
                    nc.tensor.matmul(h_ps[:, c, :], wa0[0:51, c, :],
                                     xg[p][0:51, :], start=True, stop=True)
                    nc.tensor.matmul(h_ps[:, 2 + c, :], wa0[64:115, c, :],
                                     xg[p][64:115, :], start=True, stop=True)
                hTE = s_h.tile([128, 2, TN], bf16, tag="hT")
                nc.vector.tensor_scalar(hTE[:], h_ps[:, 0:2, :], 0.0, None,
                                        ALU.max)
                hTO = s_h.tile([128, 2, TN], bf16, tag="hT")
                nc.vector.tensor_scalar(hTO[:], h_ps[:, 2:4, :], 0.0, None,
                                        ALU.max)
                hT_store[(t, 2 * p)] = hTE
                hT_store[(t, 2 * p + 1)] = hTO

            # ---- main stream
            emit_att_block(0)
            emit_att_block(1)
            emit_a0_pair(0, 0)
            for t in range(nt):
                pi = s_pi.tile([128, 4, TN], bf16, tag="pi")  # pi chunks 0..3
                ts_ = []
                for n in range(NOBJ):
                    if n % 2 == 0 and n + 2 < NOBJ:
                        emit_a0_pair(t, (n + 2) >> 1)
                    hT = hT_store.pop((t, n))
                    yA = p_y.tile([128, 2, TN], f32, tag="yA")
                    yB = p_y.tile([128, 2, TN], f32, tag="yB")
                    # odd objects fill yB first: at n==1 the yA slot is still
                    # pending the interleaved p0 relu, so lead with free banks
                    if n % 2 == 1:
                        mm_order = [(yB, 256, 384), (yB, 384, 512),
                                    (yA, 0, 128), (yA, 128, 256)]
                    else:
                        mm_order = [(yA, 0, 128), (yA, 128, 256),
                                    (yB, 256, 384), (yB, 384, 512)]
                    for kc in range(2):
                        st, sp = kc == 0, kc == 1
                        for ytile, lo_, hi_ in mm_order:
                            bank = 0 if lo_ in (0, 256) else 1
                            nc.tensor.matmul(ytile[:, bank, :],
                                             wa1[:, kc, lo_:hi_],
                                             hT[:, kc, :], start=st, stop=sp)
                    tt = s_t.tile([128, 4, TN], bf16, tag="tt")
                    if zero_ba1:
                        nc.scalar.activation(tt[:, 0:2, :], yA[:], AF.Relu)
                        nc.scalar.activation(tt[:, 2:4, :], yB[:], AF.Relu)
                    else:
                        nc.scalar.activation(tt[:, 0, :], yA[:, 0, :], AF.Relu,
                                             bias=ba1[:, 0:1])
                        nc.scalar.activation(tt[:, 1, :], yA[:, 1, :], AF.Relu,
                                             bias=ba1[:, 1:2])
                        nc.scalar.activation(tt[:, 2, :], yB[:, 0, :], AF.Relu,
                                             bias=ba1[:, 2:3])
                        nc.scalar.activation(tt[:, 3, :], yB[:, 1, :], AF.Relu,
                                             bias=ba1[:, 3:4])
                    ts_.append(tt)
                    # deepset accumulation on DVE (bf16 2x mode)
                    if n == 1:
                        nc.vector.tensor_tensor(pi[:], ts_[0][:], ts_[1][:],
                                                ALU.add)
                    elif n >= 2:
                        nc.vector.tensor_tensor(pi[:], pi[:], ts_[n][:],
                                                ALU.add)
                    # interleave the two-tile-deferred p-chain + next-tile a0s
                    if n == 1 and t >= 2:
                        emit_p0(t - 2)
                    elif n == 3 and t >= 2:
                        emit_p1(t - 2)
                    elif n == 5 and t + 1 < nt:
                        emit_a0_pair(t + 1, 0)
                    elif n == 6:
                        if t + 2 < nt:
                            emit_att_block(t + 2)
                        if t >= 2:
                            emit_p2(t - 2)
                # const-1 row for p0 bias (row 0 of last K chunk)
                nc.gpsimd.memset(pi[0:1, 3, :], 1.0)
                pi_store[t] = pi

            # ---- epilogue: p-chains for the last two tiles
            emit_p0(nt - 2)
            emit_p1(nt - 2)
            emit_p0(nt - 1)
            emit_p2(nt - 2)
            emit_p1(nt - 1)
            emit_p2(nt - 1)

    nc.compile()
    return nc


# ---------------------------------------------------------------- entry point

def _prep_in_maps(o, g, weights):
    o = np.asarray(o, np.float32)
    g = np.asarray(g, np.float32)
    in_maps = []
    for c in range(NCORES):
        sl = slice(c * BSH, (c + 1) * BSH)
        m = dict(weights)
        m.update(_pack_shard(o[sl], g[sl]))
        in_maps.append(m)
    return in_maps


def run(o, g, W_cast, b_cast, W_a0, b_a0, W_a1, b_a1,
        W_p0, b_p0, W_p1, b_p1, W_p2, b_p2, trace=False):
    from concourse.bass_utils import run_bass_kernel_spmd
    args = [np.asarray(a, np.float32) for a in
            (W_cast, b_cast, W_a0, b_a0, W_a1, b_a1, W_p0, b_p0, W_p1, b_p1,
             W_p2, b_p2)]
    weights = _pack_weights(*args)
    zero_ba1 = not np.any(args[5])
    zero_bp1 = not np.any(args[9])
    nc = build_nc(BSH, zero_ba1=zero_ba1, zero_bp1=zero_bp1)
    in_maps = _prep_in_maps(o, g, weights)
    res = run_bass_kernel_spmd(nc, in_maps, core_ids=list(range(NCORES)),
                               trace=trace)
    outs = [np.asarray(res.results[c]["out"], np.float32).T
            for c in range(NCORES)]
    return np.concatenate(outs, axis=0), res


def kernel(**inputs):
    out, _ = run(**inputs)
    return out


# revision 33
# speedup vs baseline: 1.0296x; 1.0296x over previous
# Trainium2 Bass kernel for nn_Actor (gnn_message_passing), 8-core data parallel.
#
# Math (per batch row b):
#   att = sigmoid(g @ W_cast + b_cast)                      [50]
#   x_n = concat(body(20), obj_n(30)) * att                 [50] per object n<8
#   h_n = relu(x_n @ W_a0 + b_a0)                           [256]
#   y_n = relu(h_n @ W_a1 + b_a1)                           [400]
#   pi  = sum_n y_n                                         [400]
#   out = tanh(relu(relu(pi@W_p0+b_p0)@W_p1+b_p1)@W_p2+b_p2)  [8]
#
# Mapping: everything feature-major on-chip ([feature partitions, batch free]).
# Host pre-transposes/gathers o into xsrc pair tiles (objects 2p / 2p+1 at
# partitions 0:51 / 64:115, row 50/114 is a constant-1 row that folds b_a0
# into the a0 matmul; W_cast is column-replicated so the attention PSUM
# comes out already in the gate layout). bf16 matmuls, fp32 PSUM.
import numpy as np
import ml_dtypes

BF16 = ml_dtypes.bfloat16

B = 65536
NCORES = 8
BSH = B // NCORES          # 8192 rows per core
TN = 512                   # batch tile (matmul free dim / psum bank)
BODY = 10
FEAT = 15
NOBJ = 8
HALF = 130

_BODY_COLS = list(range(0, 10)) + list(range(130, 140))


def _obj_cols(n):
    return list(range(10 + 15 * n, 25 + 15 * n)) + list(range(140 + 15 * n, 155 + 15 * n))


# ---------------------------------------------------------------- host packing

def _pack_weights(W_cast, b_cast, W_a0, b_a0, W_a1, b_a1,
                  W_p0, b_p0, W_p1, b_p1, W_p2, b_p2):
    f32 = np.float32
    # attention weights, column-replicated into the gate layout:
    # psum rows 0:50 = att[0:50] (object A gate), rows 64:114 = same (object B),
    # row 50/114 -> constant 1.0 after sigmoid (bias 30).
    wcast = np.zeros((100, 128), f32)
    wcast[:, 0:50] = W_cast
    wcast[:, 64:114] = W_cast
    bcast = np.zeros((128, 1), f32)
    bcast[0:50, 0] = b_cast
    bcast[64:114, 0] = b_cast
    bcast[50, 0] = 30.0
    bcast[114, 0] = 30.0

    # a0: K=51 (50 features + const-1 row carrying b_a0), M=256 in two chunks.
    wa0 = np.zeros((128, 2, 128), f32)
    for c in range(2):
        wa0[0:50, c, :] = W_a0[:, 128 * c:128 * (c + 1)]
        wa0[50, c, :] = b_a0[128 * c:128 * (c + 1)]
        wa0[64:114, c, :] = W_a0[:, 128 * c:128 * (c + 1)]
        wa0[114, c, :] = b_a0[128 * c:128 * (c + 1)]

    # a1: last M chunk (cols 384:512 of the packed slot) holds y[384:400] at
    # psum rows 1:17; row 0 is reserved for the p0 const-bias row.
    wa1 = np.zeros((128, 2, 512), f32)
    wa1[:, 0, 0:384] = W_a1[0:128, 0:384]
    wa1[:, 1, 0:384] = W_a1[128:256, 0:384]
    wa1[:, 0, 385:401] = W_a1[0:128, 384:400]
    wa1[:, 1, 385:401] = W_a1[128:256, 384:400]
    ba1 = np.zeros((128, 4), f32)
    ba1[:, 0] = b_a1[0:128]
    ba1[:, 1] = b_a1[128:256]
    ba1[:, 2] = b_a1[256:384]
    ba1[1:17, 3] = b_a1[384:400]

    # p0: K=400 as (128,128,128,17) — row 0 of last chunk is const-1 * b_p0.
    wp0 = np.zeros((128, 4, 256), f32)
    wp0[:, 0, :] = W_p0[0:128, :]
    wp0[:, 1, :] = W_p0[128:256, :]
    wp0[:, 2, :] = W_p0[256:384, :]
    wp0[0, 3, :] = b_p0
    wp0[1:17, 3, :] = W_p0[384:400, :]

    wp1 = np.zeros((128, 2, 256), f32)
    wp1[:, 0, :] = W_p1[0:128, :]
    wp1[:, 1, :] = W_p1[128:256, :]
    bp1 = np.zeros((128, 2), f32)
    bp1[:, 0] = b_p1[0:128]
    bp1[:, 1] = b_p1[128:256]

    wp2 = np.zeros((128, 2, 8), f32)
    wp2[:, 0, :] = W_p2[0:128, :]
    wp2[:, 1, :] = W_p2[128:256, :]
    bp2 = np.asarray(b_p2, f32).reshape(8, 1)

    return {
        "wcast": wcast.astype(BF16), "bcast": bcast,
        "wa0": wa0.astype(BF16),
        "wa1": wa1.astype(BF16), "ba1": ba1,
        "wp0": wp0.astype(BF16),
        "wp1": wp1.astype(BF16), "bp1": bp1,
        "wp2": wp2.astype(BF16), "bp2": bp2,
    }


def _pack_shard(o_sh, g_sh):
    """o_sh [bsh, 260] f32, g_sh [bsh, 100] f32 -> xsrc [4,128,bsh], gT [100,bsh] bf16."""
    bsh = o_sh.shape[0]
    oT = np.ascontiguousarray(o_sh.T)  # [260, bsh]
    xsrc = np.zeros((4, 128, bsh), np.float32)
    body = oT[_BODY_COLS]  # [20, bsh]
    for p in range(4):
        xsrc[p, 0:20] = body
        xsrc[p, 20:50] = oT[_obj_cols(2 * p)]
        xsrc[p, 50] = 1.0
        xsrc[p, 64:84] = body
        xsrc[p, 84:114] = oT[_obj_cols(2 * p + 1)]
        xsrc[p, 114] = 1.0
    gT = np.ascontiguousarray(g_sh.T)  # [100, bsh]
    return {"xsrc": xsrc.astype(BF16), "gT": gT.astype(BF16)}


# ---------------------------------------------------------------- bass kernel

def build_nc(bsh=BSH, zero_ba1=True, zero_bp1=True):
    import concourse.bass as bass
    import concourse.mybir as mybir
    import concourse.tile as tile
    from concourse import bacc

    f32 = mybir.dt.float32
    bf16 = mybir.dt.bfloat16
    AF = mybir.ActivationFunctionType
    ALU = mybir.AluOpType

    nt = bsh // TN
    nc = bacc.Bacc("TRN2", target_bir_lowering=False, debug=False)

    xsrc_d = nc.dram_tensor("xsrc", [4, 128, bsh], bf16, kind="ExternalInput")
    gT_d = nc.dram_tensor("gT", [100, bsh], bf16, kind="ExternalInput")
    wcast_d = nc.dram_tensor("wcast", [100, 128], bf16, kind="ExternalInput")
    bcast_d = nc.dram_tensor("bcast", [128, 1], f32, kind="ExternalInput")
    wa0_d = nc.dram_tensor("wa0", [128, 2, 128], bf16, kind="ExternalInput")
    wa1_d = nc.dram_tensor("wa1", [128, 2, 512], bf16, kind="ExternalInput")
    ba1_d = nc.dram_tensor("ba1", [128, 4], f32, kind="ExternalInput")
    wp0_d = nc.dram_tensor("wp0", [128, 4, 256], bf16, kind="ExternalInput")
    wp1_d = nc.dram_tensor("wp1", [128, 2, 256], bf16, kind="ExternalInput")
    bp1_d = nc.dram_tensor("bp1", [128, 2], f32, kind="ExternalInput")
    wp2_d = nc.dram_tensor("wp2", [128, 2, 8], bf16, kind="ExternalInput")
    bp2_d = nc.dram_tensor("bp2", [8, 1], f32, kind="ExternalInput")
    out_d = nc.dram_tensor("out", [8, bsh], f32, kind="ExternalOutput")

    MC = [(0, 128), (128, 128), (256, 128), (384, 16)]  # a1/p0 chunking of 400

    with tile.TileContext(nc) as tc:
        with (
            tc.tile_pool(name="s_w", bufs=1) as s_w,
            tc.tile_pool(name="s_in", bufs=3) as s_in,
            tc.tile_pool(name="s_x", bufs=2) as s_x,
            tc.tile_pool(name="s_h", bufs=5) as s_h,
            tc.tile_pool(name="s_t", bufs=4) as s_t,
            tc.tile_pool(name="s_pi", bufs=3) as s_pi,
            tc.tile_pool(name="s_p", bufs=2) as s_p,
            tc.tile_pool(name="s_o", bufs=2) as s_o,
            tc.tile_pool(name="p_h", bufs=1, space="PSUM") as p_h,
            tc.tile_pool(name="p_y", bufs=1, space="PSUM") as p_y,
        ):
            # --- load weights once
            wcast = s_w.tile([100, 128], bf16, tag="wcast")
            nc.sync.dma_start(wcast[:], wcast_d[:, :])
            bcast = s_w.tile([128, 1], f32, tag="bcast")
            nc.sync.dma_start(bcast[:], bcast_d[:, :])
            wa0 = s_w.tile([128, 2, 128], bf16, tag="wa0")
            nc.sync.dma_start(wa0[:], wa0_d[:, :, :])
            wa1 = s_w.tile([128, 2, 512], bf16, tag="wa1")
            nc.sync.dma_start(wa1[:], wa1_d[:, :, :])
            ba1 = s_w.tile([128, 4], f32, tag="ba1")
            nc.sync.dma_start(ba1[:], ba1_d[:, :])
            wp0 = s_w.tile([128, 4, 256], bf16, tag="wp0")
            nc.sync.dma_start(wp0[:], wp0_d[:, :, :])
            wp1 = s_w.tile([128, 2, 256], bf16, tag="wp1")
            nc.sync.dma_start(wp1[:], wp1_d[:, :, :])
            bp1 = s_w.tile([128, 2], f32, tag="bp1")
            nc.sync.dma_start(bp1[:], bp1_d[:, :])
            wp2 = s_w.tile([128, 2, 8], bf16, tag="wp2")
            nc.sync.dma_start(wp2[:], wp2_d[:, :, :])
            bp2 = s_w.tile([8, 1], f32, tag="bp2")
            nc.sync.dma_start(bp2[:], bp2_d[:, :])

            pi_store = {}    # t -> (piA, piB)
            p0T_store = {}   # t -> p0T
            p1T_store = {}   # t -> p1T
            xg_store = {}    # t -> [xg0..xg3]
            hT_store = {}    # (t, n) -> hT

            # ---- deferred p-chain emitters: tile t's tail is emitted during
            # ---- tile t+2, so pi/p0T/p1T latencies are fully hidden.
            def emit_p0(t):
                pi = pi_store.pop(t)
                p0_ps = p_y.tile([128, 2, TN], f32, tag="yA")
                for c in range(2):
                    mw = bass.ds(128 * c, 128)
                    nc.tensor.matmul(p0_ps[:, c, :], wp0[:, 0, mw], pi[:, 0, :],
                                     start=True, stop=False)
                    nc.tensor.matmul(p0_ps[:, c, :], wp0[:, 1, mw], pi[:, 1, :],
                                     start=False, stop=False)
                    nc.tensor.matmul(p0_ps[:, c, :], wp0[:, 2, mw], pi[:, 2, :],
                                     start=False, stop=False)
                    nc.tensor.matmul(p0_ps[:, c, :], wp0[0:17, 3, mw],
                                     pi[0:17, 3, :], start=False, stop=True)
                p0T = s_p.tile([128, 2, TN], bf16, tag="p0T")
                nc.vector.tensor_scalar(p0T[:], p0_ps[:], 0.0, None, ALU.max)
                p0T_store[t] = p0T

            def emit_p1(t):
                p0T = p0T_store.pop(t)
                p1_ps = p_h.tile([128, 2, TN], f32, tag="h")
                for c in range(2):
                    mw = bass.ds(128 * c, 128)
                    nc.tensor.matmul(p1_ps[:, c, :], wp1[:, 0, mw], p0T[:, 0, :],
                                     start=True, stop=False)
                    nc.tensor.matmul(p1_ps[:, c, :], wp1[:, 1, mw], p0T[:, 1, :],
                                     start=False, stop=True)
                p1T = s_p.tile([128, 2, TN], bf16, tag="p1T")
                if zero_bp1:
                    nc.scalar.activation(p1T[:], p1_ps[:], AF.Relu)
                else:
                    nc.scalar.activation(p1T[:, 0, :], p1_ps[:, 0, :], AF.Relu,
                                         bias=bp1[:, 0:1])
                    nc.scalar.activation(p1T[:, 1, :], p1_ps[:, 1, :], AF.Relu,
                                         bias=bp1[:, 1:2])
                p1T_store[t] = p1T

            def emit_p2(t):
                p1T = p1T_store.pop(t)
                cs = bass.ds(t * TN, TN)
                o_ps = p_h.tile([8, TN], f32, tag="h")
                nc.tensor.matmul(o_ps[:], wp2[:, 0, :], p1T[:, 0, :],
                                 start=True, stop=False)
                nc.tensor.matmul(o_ps[:], wp2[:, 1, :], p1T[:, 1, :],
                                 start=False, stop=True)
                ot = s_o.tile([8, TN], f32, tag="ot")
                nc.scalar.activation(ot[:], o_ps[:], AF.Tanh, bias=bp2[:, 0:1])
                nc.sync.dma_start(out_d[:, cs], ot[:])

            # ---- attention/gating for tile t (emitted one tile ahead so the
            # ---- att->sigmoid->gating chain never gates the object stream)
            def emit_att_block(t):
                cs = bass.ds(t * TN, TN)
                xs = []
                for p in range(4):
                    xp = s_in.tile([128, TN], bf16, tag=f"xs{p}")
                    nc.sync.dma_start(xp[:], xsrc_d[p, :, cs])
                    xs.append(xp)
                gt = s_in.tile([100, TN], bf16, tag="gt")
                nc.sync.dma_start(gt[:], gT_d[:, cs])
                att_ps = p_y.tile([128, TN], f32, tag="yB")
                nc.tensor.matmul(att_ps[:], wcast[:], gt[:], start=True,
                                 stop=True)
                attr = s_x.tile([128, TN], bf16, tag="attr")
                nc.scalar.activation(attr[:], att_ps[:], AF.Sigmoid,
                                     bias=bcast[:, 0:1])
                xg = []
                for p in range(4):
                    xgp = s_x.tile([128, TN], bf16, tag=f"xg{p}")
                    nc.gpsimd.tensor_tensor(xgp[:], xs[p][:], attr[:], ALU.mult)
                    xg.append(xgp)
                xg_store[t] = xg

            def emit_a0_pair(t, p):
                # objects 2p (rows 0:51) and 2p+1 (rows 64:115): alternate the
                # row-group halves so consecutive matmuls run concurrently on
                # disjoint PE quadrants; one 4-bank psum tile so Tile emits no
                # semaphore wait between the pair's matmuls.
                xg = xg_store[t]
                h_ps = p_h.tile([128, 4, TN], f32, tag="h")
                for c in range(2):
                    nc.tensor.matmul(h_ps[:, c, :], wa0[0:51, c, :],
                                     xg[p][0:51, :], start=True, stop=True)
                    nc.tensor.matmul(h_ps[:, 2 + c, :], wa0[64:115, c, :],
                                     xg[p][64:115, :], start=True, stop=True)
                hTE = s_h.tile([128, 2, TN], bf16, tag="hT")
                nc.vector.tensor_scalar(hTE[:], h_ps[:, 0:2, :], 0.0, None,
                                        ALU.max)
                hTO = s_h.tile([128, 2, TN], bf16, tag="hT")
                nc.vector.tensor_scalar(hTO[:], h_ps[:, 2:4, :], 0.0, None,
                                        ALU.max)
                hT_store[(t, 2 * p)] = hTE
                hT_store[(t, 2 * p + 1)] = hTO

            # ---- main stream
            emit_att_block(0)
            emit_att_block(1)
            emit_a0_pair(0, 0)
            for t in range(nt):
                pi = s_pi.tile([128, 4, TN], bf16, tag="pi")  # pi chunks 0..3
                ts_ = []
                for n in range(NOBJ):
                    if n % 2 == 0 and n + 2 < NOBJ:
                        emit_a0_pair(t, (n + 2) >> 1)
                    hT = hT_store.pop((t, n))
                    yA = p_y.tile([128, 2, TN], f32, tag="yA")
                    yB = p_y.tile([128, 2, TN], f32, tag="yB")
                    # odd objects fill yB first: at n==1 the yA slot is still
                    # pending the interleaved p0 relu, so lead with free banks
                    if n % 2 == 1:
                        mm_order = [(yB, 256, 384), (yB, 384, 512),
                                    (yA, 0, 128), (yA, 128, 256)]
                    else:
                        mm_order = [(yA, 0, 128), (yA, 128, 256),
                                    (yB, 256, 384), (yB, 384, 512)]
                    for kc in range(2):
                        st, sp = kc == 0, kc == 1
                        for ytile, lo_, hi_ in mm_order:
                            bank = 0 if lo_ in (0, 256) else 1
                            nc.tensor.matmul(ytile[:, bank, :],
                                             wa1[:, kc, lo_:hi_],
                                             hT[:, kc, :], start=st, stop=sp)
                    tt = s_t.tile([128, 4, TN], bf16, tag="tt")
                    if zero_ba1:
                        nc.scalar.activation(tt[:, 0:2, :], yA[:], AF.Relu)
                        nc.scalar.activation(tt[:, 2:4, :], yB[:], AF.Relu)
                    else:
                        nc.scalar.activation(tt[:, 0, :], yA[:, 0, :], AF.Relu,
                                             bias=ba1[:, 0:1])
                        nc.scalar.activation(tt[:, 1, :], yA[:, 1, :], AF.Relu,
                                             bias=ba1[:, 1:2])
                        nc.scalar.activation(tt[:, 2, :], yB[:, 0, :], AF.Relu,
                                             bias=ba1[:, 2:3])
                        nc.scalar.activation(tt[:, 3, :], yB[:, 1, :], AF.Relu,
                                             bias=ba1[:, 3:4])
                    ts_.append(tt)
                    # deepset accumulation on DVE (bf16 2x mode)
                    if n == 1:
                        nc.vector.tensor_tensor(pi[:], ts_[0][:], ts_[1][:],
                                                ALU.add)
                    elif n >= 2:
                        nc.vector.tensor_tensor(pi[:], pi[:], ts_[n][:],
                                                ALU.add)
                    # interleave the two-tile-deferred p-chain + next-tile a0s
                    if n == 1 and t >= 2:
                        emit_p0(t - 2)
                    elif n == 3 and t >= 2:
                        emit_p1(t - 2)
                    elif n == 5 and t + 1 < nt:
                        emit_a0_pair(t + 1, 0)
                    elif n == 6:
                        if t + 2 < nt:
                            emit_att_block(t + 2)
                        if t >= 2:
                            emit_p2(t - 2)
                # const-1 row for p0 bias (row 0 of last K chunk)
                nc.gpsimd.memset(pi[0:1, 3, :], 1.0)
                pi_store[t] = pi

            # ---- epilogue: p-chains for the last two tiles
            emit_p0(nt - 2)
            emit_p1(nt - 2)
            emit_p0(nt - 1)
            emit_p2(nt - 2)
            emit_p1(nt - 1)
            emit_p2(nt - 1)

    nc.compile()
    return nc


# ---------------------------------------------------------------- entry point

def _prep_in_maps(o, g, weights):
    o = np.asarray(o, np.float32)
    g = np.asarray(g, np.float32)
    in_maps = []
    for c in range(NCORES):
        sl = slice(c * BSH, (c + 1) * BSH)
        m = dict(weights)
        m.update(_pack_shard(o[sl], g[sl]))
        in_maps.append(m)
    return in_maps


def run(o, g, W_cast, b_cast, W_a0, b_a0, W_a1, b_a1,
        W_p0, b_p0, W_p1, b_p1, W_p2, b_p2, trace=False):
    from concourse.bass_utils import run_bass_kernel_spmd
    args = [np.asarray(a, np.float32) for a in
            (W_cast, b_cast, W_a0, b_a0, W_a1, b_a1, W_p0, b_p0, W_p1, b_p1,
             W_p2, b_p2)]
    weights = _pack_weights(*args)
    zero_ba1 = not np.any(args[5])
    zero_bp1 = not np.any(args[9])
    nc = build_nc(BSH, zero_ba1=zero_ba1, zero_bp1=zero_bp1)
    in_maps = _prep_in_maps(o, g, weights)
    res = run_bass_kernel_spmd(nc, in_maps, core_ids=list(range(NCORES)),
                               trace=trace)
    outs = [np.asarray(res.results[c]["out"], np.float32).T
            for c in range(NCORES)]
    return np.concatenate(outs, axis=0), res


def kernel(**inputs):
    out, _ = run(**inputs)
    return out
